# revision 21
# baseline (speedup 1.0000x reference)
"""Trainium2 Bass kernel for a GQA attention block (B=2, S=2048, H=2048, 32 q-heads,
8 kv-heads, head_dim 64), tensor-parallel over heads across 8 NeuronCores.

Layout strategy: everything is computed in "feature-major" (transposed) layout
[dim, seq] so no on-chip transposes of activations are needed:
  Q^T = q_w @ X^T, KV^T packed, scores S^T = K^T.T @ Q^T (k on partitions, two
  heads packed into the PE array's row groups via base-partition 0/64), exp via
  ScalarE over paired psum banks, AV via lhsT=V_nat (PE-transposed from V^T)
  with an appended ones column that accumulates the softmax denominators for
  free, o_proj partial out^T = o_w_slice @ attn^T, chunked ReduceScatter
  across cores overlapped with compute.
Host does: input transposes/slicing/bf16-cast, rope table gather (cos/sin
indexed by positions, sign-folded for the pair-swap form), final concat +
transpose.
"""

import sys

if "/opt/trn_rl_repo" not in sys.path:
    sys.path.insert(0, "/opt/trn_rl_repo")

import ml_dtypes
import numpy as np

import concourse.bass as bass
import concourse.mybir as mybir
import concourse.tile as tile
from concourse import bacc
from concourse.bass_utils import run_bass_kernel_spmd

B, S, H = 2, 2048, 2048
NH, NKV, HD = 32, 8, 64
NC = 8
HPC = NH // NC          # 4 q heads per core
QD = HPC * HD           # 256 q dims per core
NK = H // 128           # 16 contraction chunks
SC = 512                # seq chunk (matmul moving dim)
NQ = S // SC            # 4 seq chunks
G = 8                   # ReduceScatter chunks
NEG = np.float32(-1e30)
F32 = mybir.dt.float32
F32R = mybir.dt.float32r
BF16 = mybir.dt.bfloat16

MM = "bf16"             # "bf16" | "f32r"

SWAP32 = [i ^ 1 for i in range(32)]   # stream_shuffle pair-swap mask


def build(mode, mm=None, with_rs=True):
    """mode: 'zeros' | 'causal' | 'general'."""
    assert with_rs, "no-collective variant no longer supported"
    mm = mm or MM
    MMDT = BF16 if mm == "bf16" else F32R
    DRDT = BF16 if mm == "bf16" else F32

    def rcast(ap):
        # DRAM source cast for matmul-feeding tiles
        return ap if mm == "bf16" else ap.bitcast(F32R)

    nc = bacc.Bacc("TRN2", target_bir_lowering=False, debug=False, num_devices=NC)

    xT = nc.dram_tensor("xT", [B, H, S], DRDT, kind="ExternalInput").ap()
    qwT = nc.dram_tensor("qwT", [H, QD], DRDT, kind="ExternalInput").ap()
    kvwT = nc.dram_tensor("kvwT", [H, 128], DRDT, kind="ExternalInput").ap()
    qb = nc.dram_tensor("qb", [QD, 1], F32, kind="ExternalInput").ap()
    qbs = nc.dram_tensor("qbs", [QD, 1], F32, kind="ExternalInput").ap()
    kvb = nc.dram_tensor("kvb", [128, 1], F32, kind="ExternalInput").ap()
    kvbs = nc.dram_tensor("kvbs", [128, 1], F32, kind="ExternalInput").ap()
    ropeC = nc.dram_tensor("ropeC", [B, 128, S], F32, kind="ExternalInput").ap()
    ropeS = nc.dram_tensor("ropeS", [B, 128, S], F32, kind="ExternalInput").ap()
    owT = nc.dram_tensor("owT", [H, QD], DRDT, kind="ExternalInput").ap()
    ident = nc.dram_tensor("ident", [64, 64], F32, kind="ExternalInput").ap()
    if mode == "causal":
        dmask = nc.dram_tensor("dmask", [4, 128, SC], F32, kind="ExternalInput").ap()
    if mode == "general":
        maskT = nc.dram_tensor("maskT", [S, S], F32, kind="ExternalInput").ap()

    if with_rs:
        y = nc.dram_tensor("y", [B * QD, S], F32, kind="ExternalOutput").ap()
    else:
        outT_ext = nc.dram_tensor("outT", [B * H, S], F32, kind="ExternalOutput").ap()

    Exp = mybir.ActivationFunctionType.Exp
    Add = mybir.AluOpType.add
    Mult = mybir.AluOpType.mult
    Div = mybir.AluOpType.divide

    with tile.TileContext(nc) as tc:
        with (
            tc.tile_pool(name="wpool", bufs=1) as wpool,
            tc.tile_pool(name="rope", bufs=2) as rpool,
            tc.tile_pool(name="xin", bufs=3) as xpool,
            tc.tile_pool(name="acts", bufs=1) as apool,
            tc.tile_pool(name="tmp", bufs=3) as tpool,
            tc.tile_pool(name="exp", bufs=6) as epool,
            tc.tile_pool(name="norm", bufs=3) as npool,
            tc.tile_pool(name="ost", bufs=3) as opool,
            tc.tile_pool(name="psA", bufs=2, space="PSUM") as psA,
            tc.tile_pool(name="psO", bufs=2, space="PSUM") as psO,
            tc.tile_pool(name="dram", bufs=1, space="DRAM") as dpool,
            tc.tile_pool(name="mask", bufs=8) as mpool,
        ):
            # ---- constants / weights (loaded once) ----
            id_sb = wpool.tile([128, 64], F32, name="id_sb")
            nc.sync.dma_start(id_sb[64:128, :], ident[:])
            qw_sb = []
            for m in range(2):
                t = wpool.tile([128, NK, 128], MMDT, name=f"qw{m}")
                nc.sync.dma_start(
                    t[:],
                    rcast(
                        qwT.rearrange("(k p) q -> p k q", p=128)[
                            :, :, 128 * m : 128 * (m + 1)
                        ]
                    ),
                )
                qw_sb.append(t)
            kvw_sb = wpool.tile([128, NK, 128], MMDT, name="kvw_sb")
            nc.sync.dma_start(
                kvw_sb[:], rcast(kvwT.rearrange("(k p) q -> p k q", p=128))
            )
            ow_sb = wpool.tile([128, NK, QD], MMDT, name="ow_sb")
            nc.sync.dma_start(
                ow_sb[:], rcast(owT.rearrange("(k p) q -> p k q", p=128))
            )
            qb_sb, qbs_sb = [], []
            for m in range(2):
                t = wpool.tile([128, 1], F32, name=f"qb{m}")
                nc.sync.dma_start(t[:], qb[128 * m : 128 * (m + 1), :])
                qb_sb.append(t)
                t2 = wpool.tile([128, 1], F32, name=f"qbs{m}")
                nc.sync.dma_start(t2[:], qbs[128 * m : 128 * (m + 1), :])
                qbs_sb.append(t2)
            kvb_sb = wpool.tile([128, 1], F32, name="kvb_sb")
            nc.sync.dma_start(kvb_sb[:], kvb[:])
            kvbs_sb = wpool.tile([128, 1], F32, name="kvbs_sb")
            nc.sync.dma_start(kvbs_sb[:], kvbs[:])
            ones_sb = wpool.tile([128, NK], F32, name="ones_sb")
            nc.vector.memset(ones_sb[:], 1.0)
            # tiny warm-up collective: absorbs the CC init barrier and
            # cross-core launch skew while projections run
            warm_in = dpool.tile([1, 16], F32, name="warm_in")
            warm_out = dpool.tile([NC, 16], F32, name="warm_out", addr_space="Shared")
            nc.sync.dma_start(warm_in[:], ones_sb[0:1, :])
            nc.gpsimd.collective_compute(
                "AllGather",
                mybir.AluOpType.bypass,
                replica_groups=[list(range(NC))],
                ins=[warm_in.opt()],
                outs=[warm_out.opt()],
            )
            if mode == "causal":
                dm_sb = wpool.tile([128, 4, SC], F32, name="dm_sb")
                nc.sync.dma_start(dm_sb[:], dmask.rearrange("r p s -> p r s"))

            ag_out = {}
            for b in range(B):
                for n in range(NQ):
                    ag_out[(b, n)] = dpool.tile(
                        [H, SC], MMDT, name=f"ag_out{b}_{n}", addr_space="Shared"
                    )
            bt = {}

            def emit_proj(b):
                qT = [
                    apool.tile([128, S], MMDT, name=f"qT{p}", tag=f"qT{p}")
                    for p in range(2)
                ]
                sk = apool.tile([128, S], MMDT, name="sk", tag="sk")
                vnat = apool.tile([128, NK * 65], MMDT, name="vnat", tag="vnat")
                nc.vector.tensor_copy(
                    vnat.rearrange("p (j c) -> p j c", c=65)[:, :, 64:65], ones_sb[:]
                )
                attnT = [
                    apool.tile([128, S], MMDT, name=f"attnT{p}", tag=f"attnT{p}")
                    for p in range(2)
                ]
                bt[b] = dict(qT=qT, sk=sk, vnat=vnat, attnT=attnT)

                xTb = xT[b].rearrange("(k p) s -> p k s", p=128)
                for n in range(NQ):
                    nsl = slice(n * SC, (n + 1) * SC)
                    xh = []
                    for hf in range(2):
                        t = xpool.tile([128, NK // 2, SC], MMDT, name="xin", tag="xin")
                        nc.sync.dma_start(
                            t[:],
                            rcast(xTb[:, hf * (NK // 2) : (hf + 1) * (NK // 2), nsl]),
                        )
                        xh.append(t)
                    rC = rpool.tile([128, SC], F32, name="rC", tag="rC")
                    nc.sync.dma_start(rC[:], ropeC[b][:, nsl])
                    rS = rpool.tile([128, SC], F32, name="rS", tag="rS")
                    nc.sync.dma_start(rS[:], ropeS[b][:, nsl])
                    for m in range(3):
                        ps = psA.tile([128, 2, SC], F32, name="ps_proj", tag="psA")
                        for k in range(NK):
                            lhsT = qw_sb[m][:, k, :] if m < 2 else kvw_sb[:, k, :]
                            nc.tensor.matmul(
                                ps[:, 0, :], lhsT, xh[k // 8][:, k % 8, :],
                                start=(k == 0), stop=(k == NK - 1),
                            )
                        ps = ps[:, 0, :]
                        if m < 2:
                            # rope: q = (ps+qb)*C + swap(ps+qb)*S
                            xs = tpool.tile([128, SC], F32, name="xs", tag="xs")
                            nc.vector.stream_shuffle(xs[:], ps, SWAP32)
                            m1 = tpool.tile([128, SC], F32, name="m1", tag="m1")
                            nc.vector.scalar_tensor_tensor(
                                m1[:], ps, qb_sb[m][:], rC[:], op0=Add, op1=Mult
                            )
                            m2 = tpool.tile([128, SC], F32, name="m2", tag="m2")
                            nc.vector.scalar_tensor_tensor(
                                m2[:], xs[:], qbs_sb[m][:], rS[:], op0=Add, op1=Mult
                            )
                            nc.vector.tensor_add(qT[m][:, nsl], m1[:], m2[:])
                        else:
                            # K half (rows 0:64): rope -> sk[0:64], dup to [64:128]
                            xs = tpool.tile([128, SC], F32, name="xs", tag="xs")
                            nc.vector.stream_shuffle(xs[0:64, :], ps[0:64, :], SWAP32)
                            m1 = tpool.tile([128, SC], F32, name="m1", tag="m1")
                            nc.vector.scalar_tensor_tensor(
                                m1[0:64, :], ps[0:64, :], kvb_sb[0:64, :], rC[0:64, :],
                                op0=Add, op1=Mult,
                            )
                            m2 = tpool.tile([128, SC], F32, name="m2", tag="m2")
                            nc.vector.scalar_tensor_tensor(
                                m2[0:64, :], xs[0:64, :], kvbs_sb[0:64, :], rS[0:64, :],
                                op0=Add, op1=Mult,
                            )
                            nc.vector.tensor_add(sk[0:64, nsl], m1[0:64, :], m2[0:64, :])
                            nc.sync.dma_start(sk[64:128, nsl], sk[0:64, nsl])
                            # V half: bias add, then PE-transpose into vnat
                            vst = tpool.tile([128, SC], F32, name="vst", tag="vst")
                            nc.vector.tensor_scalar_add(
                                vst[64:128, :], ps[64:128, :], kvb_sb[64:128, :]
                            )
                            for js in range(4):
                                j = 4 * n + js
                                tp = psO.tile([128, 64], F32, name="tp", tag="psO")
                                nc.tensor.transpose(
                                    tp[:],
                                    vst[64:128, 128 * js : 128 * (js + 1)],
                                    id_sb[64:128, :],
                                )
                                nc.vector.tensor_copy(
                                    vnat[:, j * 65 : j * 65 + 64], tp[:]
                                )

            def emit_attn_chunk(b, n):
                qT, sk, vnat, attnT = (bt[b][k] for k in ("qT", "sk", "vnat", "attnT"))
                if True:
                    nsl = slice(n * SC, (n + 1) * SC)
                    jmax = (4 * n + 4) if mode == "causal" else NK
                    for p in range(2):
                        pO = psO.tile([65, 2, SC], F32, name="pO", tag="psO")
                        for j in range(jmax):
                            ksl = slice(j * 128, (j + 1) * 128)
                            pS = psA.tile([128, 2, SC], F32, name="pS", tag="psA")
                            nc.tensor.matmul(
                                pS[:, 0, :], sk[0:64, ksl], qT[p][0:64, nsl],
                                start=True, stop=True,
                            )
                            nc.tensor.matmul(
                                pS[:, 1, :], sk[64:128, ksl], qT[p][64:128, nsl],
                                start=True, stop=True,
                            )
                            eS = epool.tile([128, 2, SC], MMDT, name="eS", tag="eS")
                            if mode == "causal" and j >= 4 * n:
                                r = j - 4 * n
                                nc.vector.tensor_add(pS[:, 0, :], pS[:, 0, :], dm_sb[:, r, :])
                                nc.vector.tensor_add(pS[:, 1, :], pS[:, 1, :], dm_sb[:, r, :])
                                nc.scalar.activation(eS[:], pS[:], Exp, scale=0.125)
                            elif mode == "general":
                                mt = mpool.tile([128, SC], F32, name="mt", tag="mt")
                                nc.sync.dma_start(
                                    mt[:], maskT[128 * j : 128 * (j + 1), nsl]
                                )
                                nc.vector.scalar_tensor_tensor(
                                    pS[:, 0, :], pS[:, 0, :], 0.125, mt[:], op0=Mult, op1=Add
                                )
                                nc.vector.scalar_tensor_tensor(
                                    pS[:, 1, :], pS[:, 1, :], 0.125, mt[:], op0=Mult, op1=Add
                                )
                                nc.scalar.activation(eS[:], pS[:], Exp, scale=1.0)
                            else:
                                nc.scalar.activation(eS[:], pS[:], Exp, scale=0.125)
                            vsl = slice(j * 65, (j + 1) * 65)
                            nc.tensor.matmul(
                                pO[:, 0, :], vnat[:, vsl], eS[:, 0, :],
                                start=(j == 0), stop=(j == jmax - 1),
                            )
                            nc.tensor.matmul(
                                pO[:, 1, :], vnat[:, vsl], eS[:, 1, :],
                                start=(j == 0), stop=(j == jmax - 1),
                            )
                        # drain psum fast (frees the bank), then normalize from SBUF
                        pOc = npool.tile([65, 2, SC], F32, name="pOc", tag="pOc")
                        nc.vector.tensor_copy(pOc[:], pO[:])
                        den0 = npool.tile([1, 2, SC], F32, name="den0", tag="den0")
                        nc.sync.dma_start(den0[:], pOc[64:65, :, :])
                        rb = npool.tile([64, 2, SC], F32, name="rb", tag="rb")
                        nc.gpsimd.partition_broadcast(rb[:], den0[:])
                        rcp = npool.tile([64, 2, SC], F32, name="rcpb", tag="rcpb")
                        nc.vector.reciprocal_approx_fast(rcp[:], rb[:])
                        nc.vector.tensor_mul(
                            attnT[p][0:64, nsl], pOc[0:64, 0, :], rcp[:, 0, :]
                        )
                        tb = npool.tile([64, SC], MMDT, name="tb", tag="tb")
                        nc.vector.tensor_mul(tb[:], pOc[0:64, 1, :], rcp[:, 1, :])
                        nc.sync.dma_start(attnT[p][64:128, nsl], tb[:])
                    ag_in = dpool.tile(
                        [QD, SC], MMDT, name=f"ag_in{b}_{n}", tag="ag_in", bufs=4
                    )
                    for p in range(2):
                        nc.sync.dma_start(
                            ag_in[128 * p : 128 * (p + 1), :], attnT[p][:, nsl]
                        )
                    nc.gpsimd.collective_compute(
                        "AllGather",
                        mybir.AluOpType.bypass,
                        replica_groups=[list(range(NC))],
                        ins=[ag_in.opt()],
                        outs=[ag_out[(b, n)].opt()],
                    )

            def emit_oproj_chunk(b, n):
                if True:
                    nsl = slice(n * SC, (n + 1) * SC)
                    agv = ag_out[(b, n)].rearrange("(k p) s -> p k s", p=128)
                    agr = []
                    for hf in range(2):
                        t = mpool.tile(
                            [128, NK // 2, SC], MMDT, name="agr", tag="agr", bufs=3
                        )
                        nc.sync.dma_start(
                            t[:], agv[:, hf * (NK // 2) : (hf + 1) * (NK // 2), :]
                        )
                        agr.append(t)
                    for mt_i in range(2):
                        ps = psA.tile([128, 2, SC], F32, name="ps_o", tag="psA")
                        for k in range(NK):
                            nc.tensor.matmul(
                                ps[:, 0, :],
                                ow_sb[:, k, 128 * mt_i : 128 * (mt_i + 1)],
                                agr[k // 8][:, k % 8, :],
                                start=(k == 0), stop=(k == NK - 1),
                            )
                        st = opool.tile([128, SC], F32, name="st")
                        nc.vector.tensor_copy(st[:], ps[:, 0, :])
                        nc.sync.dma_start(
                            y[b * QD + 128 * mt_i : b * QD + 128 * (mt_i + 1), nsl],
                            st[:],
                        )

            # software-pipelined schedule: o_proj chunks are emitted ~2 chunks
            # behind their attention chunk so AllGather latency is hidden and
            # psum-slot allocation order never stalls ready work
            def emit_batch_attn(b):
                emit_attn_chunk(b, 0)
                emit_attn_chunk(b, 1)
                emit_oproj_chunk(b, 0)
                emit_attn_chunk(b, 2)
                emit_oproj_chunk(b, 1)
                emit_attn_chunk(b, 3)
                emit_oproj_chunk(b, 2)

            emit_proj(0)
            emit_batch_attn(0)
            emit_proj(1)
            emit_oproj_chunk(0, 3)
            emit_batch_attn(1)
            emit_oproj_chunk(1, 3)
    nc.compile()
    return nc


_cache = {}


def _get_nc(mode):
    if mode not in _cache:
        _cache[mode] = build(mode)
    return _cache[mode]


def _mode_of(mask):
    m = np.asarray(mask)
    if not np.any(m):
        return "zeros"
    m2 = m.reshape(S, S)
    tril = np.tril(np.ones((S, S), dtype=bool))
    if np.all(m2[tril] == 0.0) and np.all(m2[~tril] <= -1e30):
        return "causal"
    return "general"


def make_inputs(hidden_states, cos, sin, positions, mask, q_w, q_b, k_w, k_b,
                v_w, v_b, o_w, mode, mm=None):
    """Host-side preprocessing -> list of per-core input dicts."""
    mm = mm or MM
    ddt = ml_dtypes.bfloat16 if mm == "bf16" else np.float32
    hs = np.ascontiguousarray(np.asarray(hidden_states, dtype=np.float32))
    xT = np.ascontiguousarray(hs.transpose(0, 2, 1).astype(ddt))   # [B, H, S]
    cos = np.asarray(cos, dtype=np.float32)
    sin = np.asarray(sin, dtype=np.float32)
    pos = np.asarray(positions)
    cosg = cos[pos]                                            # [B, S, 32]
    sing = sin[pos]
    d = np.arange(64)
    idx = d % 32
    sign = np.where(d % 2 == 0, -1.0, 1.0).astype(np.float32)
    C64 = cosg[:, :, idx].transpose(0, 2, 1)                   # [B, 64, S]
    Sn64 = (sing[:, :, idx] * sign[None, None, :]).transpose(0, 2, 1)
    ropeC = np.ascontiguousarray(np.concatenate([C64, C64], axis=1))   # [B,128,S]
    ropeS = np.ascontiguousarray(np.concatenate([Sn64, Sn64], axis=1))
    ident = np.eye(64, dtype=np.float32)

    q_w = np.asarray(q_w, dtype=np.float32)
    k_w = np.asarray(k_w, dtype=np.float32)
    v_w = np.asarray(v_w, dtype=np.float32)
    o_w = np.asarray(o_w, dtype=np.float32)
    q_b = np.asarray(q_b, dtype=np.float32)
    k_b = np.asarray(k_b, dtype=np.float32)
    v_b = np.asarray(v_b, dtype=np.float32)

    extra = {}
    if mode == "causal":
        kk = np.arange(128)[:, None]
        qq = np.arange(SC)[None, :]
        extra["dmask"] = np.stack(
            [np.where(128 * r + kk <= qq, 0.0, NEG).astype(np.float32) for r in range(4)]
        )
    if mode == "general":
        extra["maskT"] = np.ascontiguousarray(
            np.asarray(mask, dtype=np.float32).reshape(S, S).T
        )

    in_maps = []
    for c in range(NC):
        qsl = slice(c * QD, (c + 1) * QD)
        ksl = slice(c * HD, (c + 1) * HD)
        qbc = q_b[qsl]
        kvb_c = np.concatenate([k_b[ksl], v_b[ksl]])
        m = {
            "xT": xT,
            "qwT": np.ascontiguousarray(q_w[qsl].T.astype(ddt)),
            "kvwT": np.ascontiguousarray(
                np.concatenate([k_w[ksl], v_w[ksl]], axis=0).T.astype(ddt)
            ),
            "qb": np.ascontiguousarray(qbc[:, None]),
            "qbs": np.ascontiguousarray(qbc[np.arange(QD) ^ 1][:, None]),
            "kvb": np.ascontiguousarray(kvb_c[:, None]),
            "kvbs": np.ascontiguousarray(kvb_c[np.arange(128) ^ 1][:, None]),
            "ropeC": ropeC,
            "ropeS": ropeS,
            "owT": np.ascontiguousarray(o_w[qsl, :].T.astype(ddt)),
            "ident": ident,
        }
        m.update(extra)
        in_maps.append(m)
    return in_maps


def assemble_output(shards):
    """shards: list of per-core y arrays [B*QD, S] -> [B, S, H] float32."""
    full = np.empty((B, H, S), dtype=np.float32)
    for c in range(NC):
        sh = shards[c].reshape(B, QD, S)
        for b in range(B):
            full[b, QD * c : QD * (c + 1)] = sh[b]
    return np.ascontiguousarray(full.transpose(0, 2, 1).astype(np.float32))


def kernel(**inputs):
    mode = _mode_of(inputs["mask"])
    nc = _get_nc(mode)
    in_maps = make_inputs(mode=mode, **{k: inputs[k] for k in (
        "hidden_states", "cos", "sin", "positions", "mask",
        "q_w", "q_b", "k_w", "k_b", "v_w", "v_b", "o_w")})
    res = run_bass_kernel_spmd(nc, in_maps, list(range(NC)))
    return assemble_output([res.results[c]["y"] for c in range(NC)])
